# revision 17
# baseline (speedup 1.0000x reference)
"""Trainium2 Bass kernel for nn_NumDualDescriptorAB (sliding-window descriptor).

Reference computation:
    X = vec_seq @ M.T                       # [S, m]
    T[w] = mean_{r<rank} X[w+r]             # sliding window mean, W = S-rank+1
    j = w % L
    scalar[w] = Bbasis[j] . T[w]
    out[w]    = Acoeff.T[j] * scalar[w]

Algebraic rewrite (matmul is linear, dot distributes over the window sum):
    C = Bbasis @ M / rank                   # [L, m]  tiny - host precompute
    scalar[w] = sum_{r} C[w%L] . v[w+r]
    out[w]    = Acoeff.T[w%L] * scalar[w]

v5 dataflow ("transposed-V"): the input is uploaded TRANSPOSED (host-side
transpose is free), so the PE contracts over the feature dim directly:

    UT[j, k] = C[ph*128+j] . v[row k]       # per 128-window tile, phase ph
    scalar[w] = sum_{k=w..w+r-1} UT[w, k]   # banded free-axis sum: ONE fused
                                            # DVE scalar_tensor_tensor with a
                                            # constant 0/1 band mask, FD=143
    out[w] = scalar[w] * AT[w%L]            # broadcast, split DVE/ACT/GPSIMD

Tiles of the same phase are processed in PAIRS: one PSUM bank holds both
UT tiles ([128, 2, 143] fp32 = 1144B/partition), and each of the 4
contraction matmuls covers both tiles as a strided moving operand
(N=2x143=286). 8 pair-banks in flight give the PE a 16-tile runway, so
matmuls stay back-to-back (probe: pipelined MMs hit ~N/2.4 ns; stalled
ones pay (219+N)/1.2). Streamed PE columns drop 44% vs the natural
layout, the DVE dot drops FD 512 -> 143.

HBM traffic is bf16 both ways (33MB/core); input blocks are single ~2MB
fully-contiguous DMAs. Sharded across 8 cores along the window axis; halo
handled host-side by overlapping shards (no collectives).
"""

import numpy as np

import concourse.bacc as bacc
import concourse.bass as bass  # noqa: F401
import concourse.mybir as mybir
import concourse.tile as tile
from concourse.bass_utils import run_bass_kernel_spmd

N_CORES = 8
M_DIM = 512
L_DIM = 512
SEQ = 131072
CHUNK = 128  # windows per tile
BC = 16  # tiles per DMA block
KK = 143  # band extent per tile (128 + max_rank - 1)
COLS_DMA = BC * CHUNK + 16  # 2064 input cols loaded per block
COLS_ALLOC = COLS_DMA  # tight alloc: all 8 block buffers fit in SBUF

_NC_CACHE = {}
_LAST_RESULTS = None  # BassKernelResults of the most recent run (for test.py)


def build_nc(nblocks: int, rank: int) -> bass.Bass:
    f32 = mybir.dt.float32
    bf16 = mybir.dt.bfloat16
    ntiles = nblocks * BC
    ncols = ntiles * CHUNK + 16

    mult = mybir.AluOpType.mult
    copy_f = mybir.ActivationFunctionType.Copy

    nc = bacc.Bacc()
    # input, transposed: v_d[s, d, row] = vec[row, 128*s + d]
    v_d = nc.dram_tensor("v", [4, CHUNK, ncols], bf16, kind="ExternalInput")
    # stationary C.T slices: ct_d[ph, s] = C[128ph:128ph+128, 128s:128s+128].T
    ct_d = nc.dram_tensor("ct", [4, 4, CHUNK, CHUNK], bf16, kind="ExternalInput")
    bm_d = nc.dram_tensor("bm", [CHUNK, KK], bf16, kind="ExternalInput")
    a_d = nc.dram_tensor("amat", [4, CHUNK, M_DIM], bf16, kind="ExternalInput")
    # output kept in device-friendly [p, c, d] order per block (fully
    # contiguous 8KB half-block partition lines); host un-permutes.
    o_d = nc.dram_tensor("o", [nblocks, CHUNK, BC, M_DIM], bf16, kind="ExternalOutput")

    with tile.TileContext(nc) as tc:
        with (
            tc.tile_pool(name="consts", bufs=1) as consts,
            tc.tile_pool(name="blocks", bufs=nblocks) as blocks,
            tc.tile_pool(name="outs", bufs=2) as outs,
            tc.tile_pool(name="psump", bufs=8, space="PSUM") as psump,
            tc.tile_pool(name="work", bufs=8) as work,
        ):
            ct16 = consts.tile([CHUNK, 4, 4, CHUNK], bf16, tag="ct16")
            nc.sync.dma_start(out=ct16, in_=ct_d[:].transpose([2, 0, 1, 3]))
            bmt = consts.tile([CHUNK, KK], bf16, tag="bm")
            nc.sync.dma_start(out=bmt, in_=bm_d[:])
            a4 = consts.tile([CHUNK, 4, M_DIM], bf16, tag="a4")
            nc.sync.dma_start(out=a4, in_=a_d[:].transpose([1, 0, 2]))

            def load_block(b):
                vt = blocks.tile([CHUNK, 4, COLS_ALLOC], bf16, tag="vt")
                nc.sync.dma_start(
                    out=vt[:, :, 0:COLS_DMA],
                    in_=v_d[
                        :, :, b * BC * CHUNK : b * BC * CHUNK + COLS_DMA
                    ].transpose([1, 0, 2]),
                )
                return vt

            # Every block gets its own buffer (16.5KB/partition x 8 fits):
            # no WAR slot recycling, so the input stream is never paced by
            # compute - all 8 loads queue upfront and run at full ring rate.
            vts = {b: load_block(b) for b in range(nblocks)}
            for b in range(nblocks):
                vt = vts[b]
                ot = outs.tile([CHUNK, BC, M_DIM], bf16, tag="ot")
                # Tiles c, c+4, c+8, c+12 share phase ph = t%4. Process in
                # pairs (c, c+4): 4 accumulating matmuls, each streaming a
                # strided [2, 143] window pair (N=286) into one PSUM bank.
                # Half-block order (tiles 0-7 then 8-15) lets each half's
                # output store issue early on the ACT HWDGE ring.
                for half in range(2):
                    for g in range(4):
                        cb = g + 8 * half
                        ph = (b * BC + cb) % 4
                        ps = psump.tile([CHUNK, 2, KK], f32, tag="ps")
                        if cb * CHUNK + 1024 <= COLS_ALLOC:
                            for s in range(4):
                                win = vt[:, s, cb * CHUNK : cb * CHUNK + 1024]
                                mv = win.rearrange("p (t x) -> p t x", t=2, x=512)[
                                    :, :, 0:KK
                                ]
                                nc.tensor.matmul(
                                    ps,
                                    ct16[:, ph, s, :],
                                    mv,
                                    start=(s == 0),
                                    stop=(s == 3),
                                )
                        else:
                            # strided pair view would exceed the tight alloc:
                            # two per-tile N=143 accumulation chains instead
                            # (still one LDWEIGHTS per s for both tiles)
                            for s in range(4):
                                for i in range(2):
                                    lo = (cb + 4 * i) * CHUNK
                                    nc.tensor.matmul(
                                        ps[:, i, :],
                                        ct16[:, ph, s, :],
                                        vt[:, s, lo : lo + KK],
                                        start=(s == 0),
                                        stop=(s == 3),
                                        skip_group_check=True,
                                    )
                        for i in range(2):
                            c = cb + 4 * i
                            sc = work.tile([CHUNK, KK], bf16, tag="sc")
                            sv = work.tile([CHUNK, 1], f32, tag="sv")
                            nc.vector.scalar_tensor_tensor(
                                out=sc,
                                in0=ps[:, i, :],
                                scalar=1.0,
                                in1=bmt,
                                op0=mult,
                                op1=mult,
                                accum_out=sv,
                            )
                            # Broadcast-engine split: measured best mix is
                            # ~2:3:2 DVE:ACT:GPSIMD (t%7) - heavier skews or
                            # clustered patterns regress (engine bursts stall
                            # the sv/psum rotation).
                            t = b * BC + c
                            m7 = t % 7
                            if m7 < 2:
                                nc.vector.tensor_scalar(
                                    out=ot[:, c, :],
                                    in0=a4[:, ph, :],
                                    scalar1=sv,
                                    scalar2=None,
                                    op0=mult,
                                )
                            elif m7 < 5:
                                nc.scalar.activation(
                                    out=ot[:, c, :],
                                    in_=a4[:, ph, :],
                                    func=copy_f,
                                    scale=sv,
                                )
                            else:
                                nc.gpsimd.tensor_tensor(
                                    ot[:, c, :],
                                    a4[:, ph, :],
                                    sv.broadcast_to([CHUNK, M_DIM]),
                                    mult,
                                )
                    nc.scalar.dma_start(
                        out=o_d[b, :, half * 8 : half * 8 + 8, :],
                        in_=ot[:, half * 8 : half * 8 + 8, :],
                    )
                del vts[b]

    nc.finalize()
    return nc


def _get_nc(nblocks: int, rank: int) -> bass.Bass:
    key = (nblocks, rank)
    if key not in _NC_CACHE:
        _NC_CACHE[key] = build_nc(nblocks, rank)
    return _NC_CACHE[key]


def kernel(vec_seq, M, Acoeff, Bbasis, rank):
    global _LAST_RESULTS
    import ml_dtypes

    bf = ml_dtypes.bfloat16
    vec_seq = np.asarray(vec_seq, dtype=np.float32)
    M = np.asarray(M, dtype=np.float32)
    Acoeff = np.asarray(Acoeff, dtype=np.float32)
    Bbasis = np.asarray(Bbasis, dtype=np.float32)
    r = int(rank)
    S, m = vec_seq.shape
    assert m == M_DIM and Bbasis.shape[0] == L_DIM
    assert 1 <= r <= 16  # band extent 127+r must fit KK=143

    W = S - r + 1
    nblocks = -(-W // (N_CORES * CHUNK * BC))
    ntiles = nblocks * BC
    nw = ntiles * CHUNK
    ncols = nw + 16

    # Transposed bf16 input, once for the full sequence: [512, S]
    vT = np.ascontiguousarray(vec_seq.astype(bf).T)

    C = ((Bbasis.astype(np.float64) @ M.astype(np.float64)) / r).astype(np.float32)
    # ct[ph, s] = C[128ph:128(ph+1), 128s:128(s+1)].T
    ct = np.ascontiguousarray(
        C.reshape(4, CHUNK, 4, CHUNK).transpose(0, 2, 3, 1)
    ).astype(bf)
    AT = np.ascontiguousarray(Acoeff.T).astype(np.float32)
    a4 = np.ascontiguousarray(AT.reshape(4, CHUNK, M_DIM)).astype(bf)
    # band mask: bm[w, k] = 1 iff w <= k <= w + r - 1
    bm = np.zeros((CHUNK, KK), dtype=np.float32)
    for w in range(CHUNK):
        bm[w, w : w + r] = 1
    bm = bm.astype(bf)

    nc = _get_nc(nblocks, r)

    in_maps = []
    for k in range(N_CORES):
        lo = k * nw
        hi = min(S, lo + ncols)
        sh = np.zeros((M_DIM, ncols), dtype=bf)
        if hi > lo:
            sh[:, : hi - lo] = vT[:, lo:hi]
        im = {
            "v": sh.reshape(4, CHUNK, ncols),
            "ct": ct,
            "bm": bm,
            "amat": a4,
        }
        in_maps.append(im)

    res = run_bass_kernel_spmd(nc, in_maps, core_ids=list(range(N_CORES)))
    _LAST_RESULTS = res
    out = np.concatenate(
        [
            res.results[k]["o"]
            .reshape(nblocks, CHUNK, BC, M_DIM)
            .transpose(0, 2, 1, 3)
            .reshape(nw, M_DIM)
            for k in range(N_CORES)
        ],
        axis=0,
    )
    return np.ascontiguousarray(out[:W].astype(np.float32))


# revision 18
# speedup vs baseline: 1.2466x; 1.2466x over previous
"""Trainium2 Bass kernel for nn_NumDualDescriptorAB (sliding-window descriptor).

Reference computation:
    X = vec_seq @ M.T                       # [S, m]
    T[w] = mean_{r<rank} X[w+r]             # sliding window mean, W = S-rank+1
    j = w % L
    scalar[w] = Bbasis[j] . T[w]
    out[w]    = Acoeff.T[j] * scalar[w]

Algebraic rewrite (matmul is linear, dot distributes over the window sum):
    C = Bbasis @ M / rank                   # [L, m]  tiny - host precompute
    scalar[w] = sum_{r} C[w%L] . v[w+r]
    out[w]    = Acoeff.T[w%L] * scalar[w]

v5 dataflow ("transposed-V"): the input is uploaded TRANSPOSED (host-side
transpose is free), so the PE contracts over the feature dim directly:

    UT[j, k] = C[ph*128+j] . v[row k]       # per 128-window tile, phase ph
    scalar[w] = sum_{k=w..w+r-1} UT[w, k]   # banded free-axis sum: ONE fused
                                            # DVE scalar_tensor_tensor with a
                                            # constant 0/1 band mask, FD=143
    out[w] = scalar[w] * AT[w%L]            # broadcast, split DVE/ACT/GPSIMD

Tiles of the same phase are processed in PAIRS: one PSUM bank holds both
UT tiles ([128, 2, 143] fp32 = 1144B/partition), and each of the 4
contraction matmuls covers both tiles as a strided moving operand
(N=2x143=286). 8 pair-banks in flight give the PE a 16-tile runway, so
matmuls stay back-to-back (probe: pipelined MMs hit ~N/2.4 ns; stalled
ones pay (219+N)/1.2). Streamed PE columns drop 44% vs the natural
layout, the DVE dot drops FD 512 -> 143.

HBM traffic is bf16 both ways (33MB/core); input blocks are single ~2MB
fully-contiguous DMAs. Sharded across 8 cores along the window axis; halo
handled host-side by overlapping shards (no collectives).
"""

import numpy as np

import concourse.bacc as bacc
import concourse.bass as bass  # noqa: F401
import concourse.mybir as mybir
import concourse.tile as tile
from concourse.bass_utils import run_bass_kernel_spmd

N_CORES = 8
M_DIM = 512
L_DIM = 512
SEQ = 131072
CHUNK = 128  # windows per tile
BC = 16  # tiles per DMA block
KK = 143  # band extent per tile (128 + max_rank - 1)
COLS_DMA = BC * CHUNK + 16  # 2064 input cols loaded per block
COLS_ALLOC = 11 * CHUNK + 1024  # 2432: AP view bound for the last pair window

_NC_CACHE = {}
_LAST_RESULTS = None  # BassKernelResults of the most recent run (for test.py)


def build_nc(nblocks: int, rank: int) -> bass.Bass:
    f32 = mybir.dt.float32
    bf16 = mybir.dt.bfloat16
    ntiles = nblocks * BC
    ncols = ntiles * CHUNK + 16

    mult = mybir.AluOpType.mult
    copy_f = mybir.ActivationFunctionType.Copy

    nc = bacc.Bacc()
    # input, transposed: v_d[s, d, row] = vec[row, 128*s + d]
    v_d = nc.dram_tensor("v", [4, CHUNK, ncols], bf16, kind="ExternalInput")
    # stationary C.T slices: ct_d[ph, s] = C[128ph:128ph+128, 128s:128s+128].T
    ct_d = nc.dram_tensor("ct", [4, 4, CHUNK, CHUNK], bf16, kind="ExternalInput")
    bm_d = nc.dram_tensor("bm", [CHUNK, KK], bf16, kind="ExternalInput")
    a_d = nc.dram_tensor("amat", [4, CHUNK, M_DIM], bf16, kind="ExternalInput")
    # output kept in device-friendly [p, c, d] order per block (fully
    # contiguous 8KB half-block partition lines); host un-permutes.
    o_d = nc.dram_tensor("o", [nblocks, CHUNK, BC, M_DIM], bf16, kind="ExternalOutput")

    with tile.TileContext(nc) as tc:
        with (
            tc.tile_pool(name="consts", bufs=1) as consts,
            tc.tile_pool(name="blocks", bufs=3) as blocks,
            tc.tile_pool(name="outs", bufs=2) as outs,
            tc.tile_pool(name="psump", bufs=8, space="PSUM") as psump,
            tc.tile_pool(name="work", bufs=8) as work,
        ):
            ct16 = consts.tile([CHUNK, 4, 4, CHUNK], bf16, tag="ct16")
            nc.sync.dma_start(out=ct16, in_=ct_d[:].transpose([2, 0, 1, 3]))
            bmt = consts.tile([CHUNK, KK], bf16, tag="bm")
            nc.sync.dma_start(out=bmt, in_=bm_d[:])
            a4 = consts.tile([CHUNK, 4, M_DIM], bf16, tag="a4")
            nc.sync.dma_start(out=a4, in_=a_d[:].transpose([1, 0, 2]))

            def load_block(b):
                vt = blocks.tile([CHUNK, 4, COLS_ALLOC], bf16, tag="vt")
                nc.sync.dma_start(
                    out=vt[:, :, 0:COLS_DMA],
                    in_=v_d[
                        :, :, b * BC * CHUNK : b * BC * CHUNK + COLS_DMA
                    ].transpose([1, 0, 2]),
                )
                return vt

            PF = 2
            vts = {b: load_block(b) for b in range(min(PF, nblocks))}
            for b in range(nblocks):
                if b + PF < nblocks:
                    vts[b + PF] = load_block(b + PF)
                vt = vts[b]
                ot = outs.tile([CHUNK, BC, M_DIM], bf16, tag="ot")
                # Tiles c, c+4, c+8, c+12 share phase ph = t%4. Process in
                # pairs (c, c+4): 4 accumulating matmuls, each streaming a
                # strided [2, 143] window pair (N=286) into one PSUM bank.
                # Half-block order (tiles 0-7 then 8-15) lets each half's
                # output store issue early on the ACT HWDGE ring.
                for half in range(2):
                    for g in range(4):
                        cb = g + 8 * half
                        ph = (b * BC + cb) % 4
                        ps = psump.tile([CHUNK, 2, KK], f32, tag="ps")
                        for s in range(4):
                            win = vt[:, s, cb * CHUNK : cb * CHUNK + 1024]
                            mv = win.rearrange("p (t x) -> p t x", t=2, x=512)[
                                :, :, 0:KK
                            ]
                            nc.tensor.matmul(
                                ps,
                                ct16[:, ph, s, :],
                                mv,
                                start=(s == 0),
                                stop=(s == 3),
                            )
                        for i in range(2):
                            c = cb + 4 * i
                            sc = work.tile([CHUNK, KK], bf16, tag="sc")
                            sv = work.tile([CHUNK, 1], f32, tag="sv")
                            nc.vector.scalar_tensor_tensor(
                                out=sc,
                                in0=ps[:, i, :],
                                scalar=1.0,
                                in1=bmt,
                                op0=mult,
                                op1=mult,
                                accum_out=sv,
                            )
                            # Broadcast-engine split: measured best mix is
                            # ~2:3:2 DVE:ACT:GPSIMD (t%7) - heavier skews or
                            # clustered patterns regress (engine bursts stall
                            # the sv/psum rotation).
                            t = b * BC + c
                            m7 = t % 7
                            if m7 < 2:
                                nc.vector.tensor_scalar(
                                    out=ot[:, c, :],
                                    in0=a4[:, ph, :],
                                    scalar1=sv,
                                    scalar2=None,
                                    op0=mult,
                                )
                            elif m7 < 5:
                                nc.scalar.activation(
                                    out=ot[:, c, :],
                                    in_=a4[:, ph, :],
                                    func=copy_f,
                                    scale=sv,
                                )
                            else:
                                nc.gpsimd.tensor_tensor(
                                    ot[:, c, :],
                                    a4[:, ph, :],
                                    sv.broadcast_to([CHUNK, M_DIM]),
                                    mult,
                                )
                    nc.scalar.dma_start(
                        out=o_d[b, :, half * 8 : half * 8 + 8, :],
                        in_=ot[:, half * 8 : half * 8 + 8, :],
                    )
                del vts[b]

    nc.finalize()
    return nc


def _get_nc(nblocks: int, rank: int) -> bass.Bass:
    key = (nblocks, rank)
    if key not in _NC_CACHE:
        _NC_CACHE[key] = build_nc(nblocks, rank)
    return _NC_CACHE[key]


def kernel(vec_seq, M, Acoeff, Bbasis, rank):
    global _LAST_RESULTS
    import ml_dtypes

    bf = ml_dtypes.bfloat16
    vec_seq = np.asarray(vec_seq, dtype=np.float32)
    M = np.asarray(M, dtype=np.float32)
    Acoeff = np.asarray(Acoeff, dtype=np.float32)
    Bbasis = np.asarray(Bbasis, dtype=np.float32)
    r = int(rank)
    S, m = vec_seq.shape
    assert m == M_DIM and Bbasis.shape[0] == L_DIM
    assert 1 <= r <= 16  # band extent 127+r must fit KK=143

    W = S - r + 1
    nblocks = -(-W // (N_CORES * CHUNK * BC))
    ntiles = nblocks * BC
    nw = ntiles * CHUNK
    ncols = nw + 16

    # Transposed bf16 input, once for the full sequence: [512, S]
    vT = np.ascontiguousarray(vec_seq.astype(bf).T)

    C = ((Bbasis.astype(np.float64) @ M.astype(np.float64)) / r).astype(np.float32)
    # ct[ph, s] = C[128ph:128(ph+1), 128s:128(s+1)].T
    ct = np.ascontiguousarray(
        C.reshape(4, CHUNK, 4, CHUNK).transpose(0, 2, 3, 1)
    ).astype(bf)
    AT = np.ascontiguousarray(Acoeff.T).astype(np.float32)
    a4 = np.ascontiguousarray(AT.reshape(4, CHUNK, M_DIM)).astype(bf)
    # band mask: bm[w, k] = 1 iff w <= k <= w + r - 1
    bm = np.zeros((CHUNK, KK), dtype=np.float32)
    for w in range(CHUNK):
        bm[w, w : w + r] = 1
    bm = bm.astype(bf)

    nc = _get_nc(nblocks, r)

    in_maps = []
    for k in range(N_CORES):
        lo = k * nw
        hi = min(S, lo + ncols)
        sh = np.zeros((M_DIM, ncols), dtype=bf)
        if hi > lo:
            sh[:, : hi - lo] = vT[:, lo:hi]
        im = {
            "v": sh.reshape(4, CHUNK, ncols),
            "ct": ct,
            "bm": bm,
            "amat": a4,
        }
        in_maps.append(im)

    res = run_bass_kernel_spmd(nc, in_maps, core_ids=list(range(N_CORES)))
    _LAST_RESULTS = res
    out = np.concatenate(
        [
            res.results[k]["o"]
            .reshape(nblocks, CHUNK, BC, M_DIM)
            .transpose(0, 2, 1, 3)
            .reshape(nw, M_DIM)
            for k in range(N_CORES)
        ],
        axis=0,
    )
    return np.ascontiguousarray(out[:W].astype(np.float32))
